# revision 7
# baseline (speedup 1.0000x reference)
"""Trainium2 Bass kernel for efficient-attention (nn_Attention_65532611003000).

Sharding: data-parallel over batch. B == n_cores == 8, so core i processes
batch element i end-to-end; no collectives are needed.

v3 design: host-transposed fp8 activations, fp8 DoubleRow matmuls everywhere
the precision budget allows, k-major pass 2 software-pipelined by one group.

Host prep (part of kernel(), like the weight rearrangement): x and y are
transposed to channel-major and quantized to fp8 e4m3 once on the host;
weights are pre-quantized (Wk/Wv/Wq fp8, Wr bf16) and pre-tiled.

Per-core math ([Nt, Ch] = [4096, 512] activations, H=8 heads, 64 ch/head):
  pass 1 (per 128-token tile):
    kpre = xT8'@Wk8 + yT8'@Wk8              # fp8 DoubleRow (256-deep)
    khat = fp8(exp(kpre))                   # bk drops out of token-softmax
    vpre = xT8'@Wv8                         # fp8 DoubleRow
    S_t += [khat|khat']_t' @ [vpre|1]_t     # fp8 DoubleRow over tile pairs
  epilogue: ctx_t = S_t * (1/Zk) + bv       # blockdiag per 2 heads
  pass 2 (per 512-token group, pipelined by one group):
    qpreT = Wq8' @ yT8                      # fp8 DoubleRow, transposed out
    u = bf16(exp(qpreT + bq))               # bq as per-partition bias
    den = sel' @ u                          # per-head token sums (matmul)
    denB_t = selT' @ bf16(1/den)            # broadcast to v partitions
    att_t = bf16((ctx_t^T @ u_t) * denB_t)  # unnormalized attend, then mul
    out_j = sum_t att_t[:, j]' @ Wr16_t + br
"""

import sys

sys.path.insert(0, "/opt/trn_rl_repo")

import numpy as np
import ml_dtypes
from contextlib import ExitStack

import concourse.bass as bass
import concourse.bacc as bacc
import concourse.mybir as mybir
import concourse.tile as tile
from concourse.bass_utils import run_bass_kernel_spmd

B, Nt, Ch = 8, 4096, 512
H, HK = 8, 64
P = 128            # token chunk rows / SBUF partitions
NT = Nt // P       # 32 token tiles
CT = Ch // P       # 4 channel blocks
GRP = 4            # pass-2 tiles per group (512 tokens)
NG = NT // GRP     # 8 groups

F32 = mybir.dt.float32
BF16 = mybir.dt.bfloat16
F8 = mybir.dt.float8e4
AF = mybir.ActivationFunctionType
DR = mybir.MatmulPerfMode.DoubleRow

BF16_NP = ml_dtypes.bfloat16
F8_NP = ml_dtypes.float8_e4m3


def build_nc():
    nc = bacc.Bacc(None)

    # activations, host-transposed to channel-major fp8
    # xT8/yT8: [p, tile, blk, tok] = x[128*tile+tok, 128*blk+p]  (pass 1)
    # yT8b:    [p, blk, tile, tok]                               (pass 2)
    xt_d = nc.declare_dram_parameter("xT8", [P, NT * CT * P], F8, isOutput=False)
    yt_d = nc.declare_dram_parameter("yT8", [P, NT * CT * P], F8, isOutput=False)
    ytb_d = nc.declare_dram_parameter("yT8b", [P, CT * NT * P], F8, isOutput=False)
    wk_d = nc.declare_dram_parameter("wk8", [P, 4 * Ch], F8, isOutput=False)
    wv_d = nc.declare_dram_parameter("wv8", [P, 4 * Ch], F8, isOutput=False)
    wq_d = nc.declare_dram_parameter("wq8", [P, 4 * Ch], F8, isOutput=False)
    wr_d = nc.declare_dram_parameter("wr16", [P, CT * Ch], BF16, isOutput=False)
    bqc_d = nc.declare_dram_parameter("bq_col", [P, CT], F32, isOutput=False)
    seld_d = nc.declare_dram_parameter("sel_den", [P, CT * H], BF16, isOutput=False)
    selt_d = nc.declare_dram_parameter("selT_bc", [H, CT * P], BF16, isOutput=False)
    bvb_d = nc.declare_dram_parameter("bv_blk", [P, Ch], F32, isOutput=False)
    brb_d = nc.declare_dram_parameter("br_bcast", [P, Ch], F32, isOutput=False)
    ones_d = nc.declare_dram_parameter("ones_col", [P, 2 * CT * 2], F8, isOutput=False)
    out_d = nc.declare_dram_parameter("out", [Nt, Ch], F32, isOutput=True)

    xt_v = xt_d[:].rearrange("p (t b q) -> p t b q", t=NT, b=CT)
    yt_v = yt_d[:].rearrange("p (t b q) -> p t b q", t=NT, b=CT)
    ytb_v = ytb_d[:].rearrange("p (b t q) -> p b t q", b=CT, t=NT)

    with tile.TileContext(nc) as tc, ExitStack() as ctx:
        const = ctx.enter_context(tc.tile_pool(name="const", bufs=1))

        wk8 = const.tile([P, 2, 2, Ch], F8)
        wv8 = const.tile([P, 2, 2, Ch], F8)
        wq8 = const.tile([P, 2, 2, CT, P], F8)
        wr16 = const.tile([P, CT, Ch], BF16)
        bq_col = const.tile([P, CT], F32)
        sel_den = const.tile([P, CT, H], BF16)
        selT = const.tile([H, CT, P], BF16)
        bvb = const.tile([P, Ch], F32)
        brb = const.tile([P, Ch], F32)
        ctxR = const.tile([P, CT, P], BF16)      # per-head ctx, blockdiag
        zkinv = const.tile([P, CT], F32)

        nc.sync.dma_start(wk8[:], wk_d[:].rearrange("p (g i o) -> p g i o", g=2, i=2))
        nc.sync.dma_start(wv8[:], wv_d[:].rearrange("p (g i o) -> p g i o", g=2, i=2))
        nc.sync.dma_start(
            wq8[:], wq_d[:].rearrange("p (g i kb m) -> p g i kb m", g=2, i=2, kb=CT)
        )  # pass-2 consts follow
        nc.sync.dma_start(wr16[:], wr_d[:].rearrange("p (t o) -> p t o", t=CT))
        nc.sync.dma_start(bq_col[:], bqc_d[:])
        nc.sync.dma_start(sel_den[:], seld_d[:].rearrange("p (t h) -> p t h", t=CT))
        nc.sync.dma_start(selT[:], selt_d[:].rearrange("p (t m) -> p t m", t=CT))
        nc.sync.dma_start(bvb[:], bvb_d[:])
        nc.sync.dma_start(brb[:], brb_d[:])

        # ---------------- pass 1: khat, v, S & Zk accumulation --------------
        with (
            tc.tile_pool(name="io1", bufs=3) as io1,
            tc.tile_pool(name="sb1", bufs=2) as sb1,
            tc.tile_pool(name="ps_k", bufs=2, space="PSUM") as ps_k,
            tc.tile_pool(name="ps_v", bufs=2, space="PSUM") as ps_v,
            tc.tile_pool(name="ps_s", bufs=1, space="PSUM") as ps_s,
        ):
            s_acc = [
                ps_s.tile([P, 130], F32, tag=f"sacc{t}", name=f"sacc{t}")
                for t in range(CT)
            ]
            v_aug_bufs = [
                sb1.tile([P, 2, CT, 130], F8, tag=f"vaug{n}", name=f"vaug{n}")
                for n in range(2)
            ]
            for n in range(2):
                nc.sync.dma_start(
                    v_aug_bufs[n][:, :, :, 128:130],
                    ones_d[:].rearrange("p (i t c) -> p i t c", i=2, t=CT),
                )
            khat8 = None

            for i in range(NT):
                xt = io1.tile([P, CT, P], F8, tag="xt")
                yt = io1.tile([P, CT, P], F8, tag="yt")
                nc.sync.dma_start(xt[:], xt_v[:, i, :, :])
                nc.sync.dma_start(yt[:], yt_v[:, i, :, :])

                kpre = ps_k.tile([P, Ch], F32, tag="kpre")
                for g in range(2):
                    nc.tensor.matmul(
                        kpre[:],
                        xt[:, 2 * g : 2 * g + 2, :],
                        wk8[:, g, :, :],
                        start=(g == 0),
                        stop=False,
                        perf_mode=DR,
                    )
                for g in range(2):
                    nc.tensor.matmul(
                        kpre[:],
                        yt[:, 2 * g : 2 * g + 2, :],
                        wk8[:, g, :, :],
                        start=False,
                        stop=(g == 1),
                        perf_mode=DR,
                    )
                khat = sb1.tile([P, Ch], BF16, tag="khat", name="khat")
                nc.scalar.activation(khat[:], kpre[:], AF.Exp)
                if i % 2 == 0:
                    khat8 = sb1.tile([P, 2, Ch], F8, tag="khat8", name="khat8")
                nc.gpsimd.tensor_copy(khat8[:, i % 2, :], khat[:])

                vpre = ps_v.tile([P, Ch], F32, tag="vpre")
                for g in range(2):
                    nc.tensor.matmul(
                        vpre[:],
                        xt[:, 2 * g : 2 * g + 2, :],
                        wv8[:, g, :, :],
                        start=(g == 0),
                        stop=(g == 1),
                        perf_mode=DR,
                    )
                v_aug = v_aug_bufs[(i // 2) % 2]
                nc.vector.tensor_copy(
                    v_aug[:, i % 2, :, 0:128],
                    vpre[:].rearrange("p (t q) -> p t q", t=CT),
                )

                if i % 2 == 1:
                    for t in range(CT):
                        nc.tensor.matmul(
                            s_acc[t][:],
                            khat8[:, :, P * t : P * (t + 1)],
                            v_aug[:, :, t, :],
                            start=(i == 1),
                            stop=(i == NT - 1),
                            perf_mode=DR,
                        )

            # ------------- epilogue: ctx = S * zkinv + bv ------------------
            zk4 = sb1.tile([P, CT], F32, name="zk4", tag="zk4")
            for t in range(CT):
                nc.vector.tensor_copy(zk4[:, t : t + 1], s_acc[t][:, 128:129])
            zkln = sb1.tile([P, CT], F32, name="zkln", tag="zkln")
            nc.scalar.activation(zkln[:], zk4[:], AF.Ln)
            nc.scalar.activation(zkinv[:], zkln[:], AF.Exp, scale=-1.0)
            for t in range(CT):
                nc.vector.tensor_copy(ctxR[:, t, :], bvb[:, P * t : P * (t + 1)])
                for blk in range(2):
                    p0 = 64 * blk
                    nc.vector.scalar_tensor_tensor(
                        ctxR[p0 : p0 + 64, t, p0 : p0 + 64],
                        s_acc[t][p0 : p0 + 64, p0 : p0 + 64],
                        zkinv[p0 : p0 + 64, t : t + 1],
                        bvb[p0 : p0 + 64, P * t + p0 : P * t + p0 + 64],
                        op0=mybir.AluOpType.mult,
                        op1=mybir.AluOpType.add,
                    )

        # -------- pass 2: q softmax, attend, reproject (1-group pipeline) ----
        with (
            tc.tile_pool(name="io2", bufs=3) as io2,
            tc.tile_pool(name="sb2", bufs=2) as sb2,
            tc.tile_pool(name="ps_q", bufs=2, space="PSUM") as ps_q,
            tc.tile_pool(name="ps_dd", bufs=2, space="PSUM") as ps_dd,
            tc.tile_pool(name="ps_n", bufs=2, space="PSUM") as ps_n,
            tc.tile_pool(name="ps_o", bufs=2, space="PSUM") as ps_o,
        ):
            u_prev = None
            dinv_prev = None
            for gg in range(NG + 1):
                # stage 1: q projection + exp for group gg
                if gg < NG:
                    j0 = GRP * gg
                    yq = io2.tile([P, CT, GRP * P], F8, tag="yq")
                    nc.sync.dma_start(yq[:], ytb_v[:, :, j0 : j0 + GRP, :])
                    u = sb2.tile([P, CT, Ch], BF16, tag="u")
                    for kb in range(CT):
                        qk = ps_q.tile([P, Ch], F32, tag="qk")
                        for g in range(2):
                            nc.tensor.matmul(
                                qk[:],
                                wq8[:, g, :, kb, :],
                                yq[:, 2 * g : 2 * g + 2, :],
                                start=(g == 0),
                                stop=(g == 1),
                                perf_mode=DR,
                            )
                        nc.scalar.activation(
                            u[:, kb, :], qk[:], AF.Exp, bias=bq_col[:, kb : kb + 1]
                        )

                # stage 2: denB broadcast, attend, normalize for group gg-1
                if gg > 0:
                    att = sb2.tile([P, CT, Ch], BF16, tag="att")
                    for t in range(CT):
                        db = ps_dd.tile([P, Ch], F32, tag="dd")
                        nc.tensor.matmul(
                            db[:], selT[:, t, :], dinv_prev[:], start=True, stop=True
                        )
                        dbs = sb2.tile([P, Ch], BF16, tag="dbs")
                        nc.scalar.copy(dbs[:], db[:])
                        num = ps_n.tile([P, Ch], F32, tag="num")
                        nc.tensor.matmul(
                            num[:],
                            ctxR[:, t, :],
                            u_prev[:, t, :],
                            start=True,
                            stop=True,
                        )
                        nc.vector.tensor_mul(att[:, t, :], num[:], dbs[:])

                # stage 3: den sums + reciprocal (via scalar ln/exp) for group gg
                if gg < NG:
                    dden = ps_n.tile([P, Ch], F32, tag="num", name="dden")
                    for kb in range(CT):
                        nc.tensor.matmul(
                            dden[0:H, :],
                            sel_den[:, kb, :],
                            u[:, kb, :],
                            start=(kb == 0),
                            stop=(kb == CT - 1),
                        )
                    dln = sb2.tile([H, Ch], F32, tag="dln")
                    nc.scalar.activation(dln[:], dden[0:H, :], AF.Ln)
                    deninv = sb2.tile([H, Ch], BF16, tag="dinv")
                    nc.scalar.activation(deninv[:], dln[:], AF.Exp, scale=-1.0)

                # stage 4: reproject + bias + store for group gg-1
                if gg > 0:
                    for j in range(GRP):
                        i = GRP * (gg - 1) + j
                        opre = ps_o.tile([P, Ch], F32, tag="opre")
                        for t in range(CT):
                            nc.tensor.matmul(
                                opre[:],
                                att[:, t, P * j : P * (j + 1)],
                                wr16[:, t, :],
                                start=(t == 0),
                                stop=(t == CT - 1),
                            )
                        o_sb = io2.tile([P, Ch], F32, tag="osb")
                        nc.vector.tensor_add(o_sb[:], opre[:], brb[:])
                        nc.sync.dma_start(out_d[P * i : P * (i + 1), :], o_sb[:])

                if gg < NG:
                    u_prev = u
                    dinv_prev = deninv

    nc.finalize()
    return nc


def _host_consts(Wk, bk, Wq, bq, Wv, bv, Wr, br):
    def w8(w):
        # [p, g, i, o] = W[256g + 128i + p, o]
        return np.ascontiguousarray(
            w.reshape(2, 2, P, Ch).transpose(2, 0, 1, 3).reshape(P, 4 * Ch)
        ).astype(F8_NP)

    # wq8: [p, g, i, kb, m] = Wq[256g + 128i + p, 128kb + m]
    wq8 = np.ascontiguousarray(
        Wq.reshape(2, 2, P, CT, P).transpose(2, 0, 1, 3, 4).reshape(P, 4 * Ch)
    ).astype(F8_NP)
    wr16 = np.ascontiguousarray(
        Wr.reshape(CT, P, Ch).transpose(1, 0, 2).reshape(P, CT * Ch)
    ).astype(BF16_NP)
    bq_col = np.ascontiguousarray(bq.reshape(CT, P).T).astype(np.float32)

    sel_den = np.zeros((P, CT, H), np.float32)
    for kb in range(CT):
        sel_den[0:64, kb, 2 * kb] = 1.0
        sel_den[64:128, kb, 2 * kb + 1] = 1.0

    selT_bc = np.zeros((H, CT, P), np.float32)
    for t in range(CT):
        selT_bc[2 * t, t, 0:64] = 1.0
        selT_bc[2 * t + 1, t, 64:128] = 1.0

    bvb = np.zeros((P, Ch), np.float32)
    for t in range(CT):
        for blk in range(2):
            p0 = 64 * blk
            c0 = P * t + p0
            bvb[p0 : p0 + 64, c0 : c0 + 64] = bv[None, c0 : c0 + 64]

    return {
        "wk8": w8(Wk),
        "wv8": w8(Wv),
        "wq8": wq8,
        "wr16": wr16,
        "bq_col": bq_col,
        "sel_den": sel_den.reshape(P, CT * H).astype(BF16_NP),
        "selT_bc": np.ascontiguousarray(selT_bc.reshape(H, CT * P)).astype(BF16_NP),
        "bv_blk": bvb,
        "br_bcast": np.ascontiguousarray(np.tile(br[None, :], (P, 1))).astype(
            np.float32
        ),
        "ones_col": np.ones((P, 2 * CT * 2), F8_NP),
    }


_NC_CACHE = {}


def _get_nc():
    if "nc" not in _NC_CACHE:
        _NC_CACHE["nc"] = build_nc()
    return _NC_CACHE["nc"]


def kernel(input_, y, Wk, bk, Wq, bq, Wv, bv, Wr, br, _trace=False, _tmpdir=None):
    input_ = np.asarray(input_, np.float32)
    y = np.asarray(y, np.float32)
    consts = _host_consts(
        np.asarray(Wk, np.float32), np.asarray(bk, np.float32),
        np.asarray(Wq, np.float32), np.asarray(bq, np.float32),
        np.asarray(Wv, np.float32), np.asarray(bv, np.float32),
        np.asarray(Wr, np.float32), np.asarray(br, np.float32),
    )

    def tile_major(a8):
        # [p, tile, blk, tok] = a[128*tile+tok, 128*blk+p]
        return np.ascontiguousarray(
            a8.reshape(NT, P, CT, P).transpose(3, 0, 2, 1).reshape(P, NT * CT * P)
        )

    def blk_major(a8):
        # [p, blk, tile, tok]
        return np.ascontiguousarray(
            a8.reshape(NT, P, CT, P).transpose(3, 2, 0, 1).reshape(P, CT * NT * P)
        )

    nc = _get_nc()
    in_maps = []
    for i in range(B):
        x8 = input_[i].astype(F8_NP)
        y8 = y[i].astype(F8_NP)
        in_maps.append(
            {
                "xT8": tile_major(x8),
                "yT8": tile_major(y8),
                "yT8b": blk_major(y8),
                **consts,
            }
        )
    res = run_bass_kernel_spmd(
        nc, in_maps, core_ids=list(range(B)), trace=_trace, tmpdir=_tmpdir
    )
    out = np.stack([res.results[i]["out"] for i in range(B)], axis=0)
    if _trace:
        return out, res
    return out


# revision 9
# speedup vs baseline: 1.2043x; 1.2043x over previous
"""Trainium2 Bass kernel for efficient-attention (nn_Attention_65532611003000).

Sharding: data-parallel over batch. B == n_cores == 8, so core i processes
batch element i end-to-end; no collectives are needed.

v3 design: host-transposed fp8 activations, fp8 DoubleRow matmuls everywhere
the precision budget allows, k-major pass 2 software-pipelined by one group.

Host prep (part of kernel(), like the weight rearrangement): x and y are
transposed to channel-major and quantized to fp8 e4m3 once on the host;
weights are pre-quantized (Wk/Wv/Wq fp8, Wr bf16) and pre-tiled.

Per-core math ([Nt, Ch] = [4096, 512] activations, H=8 heads, 64 ch/head):
  pass 1 (per 128-token tile):
    kpre = xT8'@Wk8 + yT8'@Wk8              # fp8 DoubleRow (256-deep)
    khat = fp8(exp(kpre))                   # bk drops out of token-softmax
    vpre = xT8'@Wv8                         # fp8 DoubleRow
    S_t += [khat|khat']_t' @ [vpre|1]_t     # fp8 DoubleRow over tile pairs
  epilogue: ctx_t = S_t * (1/Zk) + bv       # blockdiag per 2 heads
  pass 2 (per 512-token group, pipelined by one group):
    qpreT = Wq8' @ yT8                      # fp8 DoubleRow, transposed out
    u = bf16(exp(qpreT + bq))               # bq as per-partition bias
    den = sel' @ u                          # per-head token sums (matmul)
    denB_t = selT' @ bf16(1/den)            # broadcast to v partitions
    att_t = bf16((ctx_t^T @ u_t) * denB_t)  # unnormalized attend, then mul
    out_j = sum_t att_t[:, j]' @ Wr16_t + br
"""

import sys

sys.path.insert(0, "/opt/trn_rl_repo")

import numpy as np
import ml_dtypes
from contextlib import ExitStack

import concourse.bass as bass
import concourse.bacc as bacc
import concourse.mybir as mybir
import concourse.tile as tile
from concourse.bass_utils import run_bass_kernel_spmd

B, Nt, Ch = 8, 4096, 512
H, HK = 8, 64
P = 128            # token chunk rows / SBUF partitions
NT = Nt // P       # 32 token tiles
CT = Ch // P       # 4 channel blocks
GRP = 4            # pass-2 tiles per group (512 tokens)
NG = NT // GRP     # 8 groups

F32 = mybir.dt.float32
BF16 = mybir.dt.bfloat16
F8 = mybir.dt.float8e4
AF = mybir.ActivationFunctionType
DR = mybir.MatmulPerfMode.DoubleRow

BF16_NP = ml_dtypes.bfloat16
F8_NP = ml_dtypes.float8_e4m3


def build_nc():
    nc = bacc.Bacc(None)

    # activations, host-transposed to channel-major fp8
    # xT8/yT8: [p, tile, blk, tok] = x[128*tile+tok, 128*blk+p]  (pass 1)
    # yT8b:    [p, blk, tile, tok]                               (pass 2)
    xt_d = nc.declare_dram_parameter("xT8", [P, NT * CT * P], F8, isOutput=False)
    yt_d = nc.declare_dram_parameter("yT8", [P, NT * CT * P], F8, isOutput=False)
    ytb_d = nc.declare_dram_parameter("yT8b", [P, CT * NT * P], F8, isOutput=False)
    wk_d = nc.declare_dram_parameter("wk8", [P, 4 * Ch], F8, isOutput=False)
    wv_d = nc.declare_dram_parameter("wv8", [P, 4 * Ch], F8, isOutput=False)
    wq_d = nc.declare_dram_parameter("wq8", [P, 4 * Ch], F8, isOutput=False)
    wr_d = nc.declare_dram_parameter("wr16", [P, CT * Ch], BF16, isOutput=False)
    bqc_d = nc.declare_dram_parameter("bq_col", [P, CT], F32, isOutput=False)
    seld_d = nc.declare_dram_parameter("sel_den", [P, CT * H], BF16, isOutput=False)
    selt_d = nc.declare_dram_parameter("selT_bc", [H, CT * P], BF16, isOutput=False)
    bvb_d = nc.declare_dram_parameter("bv_blk", [P, Ch], F32, isOutput=False)
    brb_d = nc.declare_dram_parameter("br_bcast", [P, Ch], F32, isOutput=False)
    ones_d = nc.declare_dram_parameter("ones_col", [P, 2 * CT * 2], BF16, isOutput=False)
    out_d = nc.declare_dram_parameter("out", [Nt, Ch], F32, isOutput=True)

    xt_v = xt_d[:].rearrange("p (t b q) -> p t b q", t=NT, b=CT)
    yt_v = yt_d[:].rearrange("p (t b q) -> p t b q", t=NT, b=CT)
    ytb_v = ytb_d[:].rearrange("p (b t q) -> p b t q", b=CT, t=NT)

    with tile.TileContext(nc) as tc, ExitStack() as ctx:
        const = ctx.enter_context(tc.tile_pool(name="const", bufs=1))

        wk8 = const.tile([P, 2, 2, Ch], F8)
        wv8 = const.tile([P, 2, 2, Ch], F8)
        wq8 = const.tile([P, 2, 2, CT, P], F8)
        wr16 = const.tile([P, CT, Ch], BF16)
        bq_col = const.tile([P, CT], F32)
        sel_den = const.tile([P, CT, H], BF16)
        selT = const.tile([H, CT, P], BF16)
        bvb = const.tile([P, Ch], F32)
        brb = const.tile([P, Ch], F32)
        ctxR = const.tile([P, CT, P], BF16)      # per-head ctx, blockdiag
        zkinv = const.tile([P, CT], F32)

        nc.sync.dma_start(wk8[:], wk_d[:].rearrange("p (g i o) -> p g i o", g=2, i=2))
        nc.sync.dma_start(wv8[:], wv_d[:].rearrange("p (g i o) -> p g i o", g=2, i=2))
        nc.sync.dma_start(
            wq8[:], wq_d[:].rearrange("p (g i kb m) -> p g i kb m", g=2, i=2, kb=CT)
        )  # pass-2 consts follow
        nc.sync.dma_start(wr16[:], wr_d[:].rearrange("p (t o) -> p t o", t=CT))
        nc.sync.dma_start(bq_col[:], bqc_d[:])
        nc.sync.dma_start(sel_den[:], seld_d[:].rearrange("p (t h) -> p t h", t=CT))
        nc.sync.dma_start(selT[:], selt_d[:].rearrange("p (t m) -> p t m", t=CT))
        nc.sync.dma_start(bvb[:], bvb_d[:])
        nc.sync.dma_start(brb[:], brb_d[:])

        # ---------------- pass 1: khat, v, S & Zk accumulation --------------
        with (
            tc.tile_pool(name="io1", bufs=3) as io1,
            tc.tile_pool(name="sb1", bufs=2) as sb1,
            tc.tile_pool(name="ps_k", bufs=2, space="PSUM") as ps_k,
            tc.tile_pool(name="ps_v", bufs=2, space="PSUM") as ps_v,
            tc.tile_pool(name="ps_s", bufs=1, space="PSUM") as ps_s,
        ):
            s_acc = [
                ps_s.tile([P, 130], F32, tag=f"sacc{t}", name=f"sacc{t}")
                for t in range(CT)
            ]
            v_aug_bufs = [
                sb1.tile([P, CT, 130], BF16, tag=f"vaug{n}", name=f"vaug{n}")
                for n in range(2)
            ]
            for n in range(2):
                nc.sync.dma_start(
                    v_aug_bufs[n][:, :, 128:130],
                    ones_d[:, 0 : CT * 2].rearrange("p (t c) -> p t c", t=CT),
                )

            for i in range(NT):
                xt = io1.tile([P, CT, P], F8, tag="xt")
                yt = io1.tile([P, CT, P], F8, tag="yt")
                nc.gpsimd.dma_start(xt[:], xt_v[:, i, :, :])
                nc.gpsimd.dma_start(yt[:], yt_v[:, i, :, :])

                kpre = ps_k.tile([P, Ch], F32, tag="kpre")
                for g in range(2):
                    nc.tensor.matmul(
                        kpre[:],
                        xt[:, 2 * g : 2 * g + 2, :],
                        wk8[:, g, :, :],
                        start=(g == 0),
                        stop=False,
                        perf_mode=DR,
                    )
                for g in range(2):
                    nc.tensor.matmul(
                        kpre[:],
                        yt[:, 2 * g : 2 * g + 2, :],
                        wk8[:, g, :, :],
                        start=False,
                        stop=(g == 1),
                        perf_mode=DR,
                    )
                khat = sb1.tile([P, Ch], BF16, tag="khat", name="khat")
                nc.scalar.activation(khat[:], kpre[:], AF.Exp)

                vpre = ps_v.tile([P, Ch], F32, tag="vpre")
                for g in range(2):
                    nc.tensor.matmul(
                        vpre[:],
                        xt[:, 2 * g : 2 * g + 2, :],
                        wv8[:, g, :, :],
                        start=(g == 0),
                        stop=(g == 1),
                        perf_mode=DR,
                    )
                v_aug = v_aug_bufs[i % 2]
                nc.vector.tensor_copy(
                    v_aug[:, :, 0:128],
                    vpre[:].rearrange("p (t q) -> p t q", t=CT),
                )

                for t in range(CT):
                    nc.tensor.matmul(
                        s_acc[t][:],
                        khat[:, P * t : P * (t + 1)],
                        v_aug[:, t, :],
                        start=(i == 0),
                        stop=(i == NT - 1),
                    )

            # ------------- epilogue: ctx = S * zkinv + bv ------------------
            zk4 = sb1.tile([P, CT], F32, name="zk4", tag="zk4")
            for t in range(CT):
                nc.vector.tensor_copy(zk4[:, t : t + 1], s_acc[t][:, 128:129])
            nc.vector.reciprocal(zkinv[:], zk4[:])
            for t in range(CT):
                nc.vector.tensor_copy(ctxR[:, t, :], bvb[:, P * t : P * (t + 1)])
                for blk in range(2):
                    p0 = 64 * blk
                    nc.vector.scalar_tensor_tensor(
                        ctxR[p0 : p0 + 64, t, p0 : p0 + 64],
                        s_acc[t][p0 : p0 + 64, p0 : p0 + 64],
                        zkinv[p0 : p0 + 64, t : t + 1],
                        bvb[p0 : p0 + 64, P * t + p0 : P * t + p0 + 64],
                        op0=mybir.AluOpType.mult,
                        op1=mybir.AluOpType.add,
                    )

        # -------- pass 2: q softmax, attend, reproject (1-group pipeline) ----
        with (
            tc.tile_pool(name="io2", bufs=3) as io2,
            tc.tile_pool(name="sb2", bufs=2) as sb2,
            tc.tile_pool(name="ps_q", bufs=2, space="PSUM") as ps_q,
            tc.tile_pool(name="ps_dd", bufs=2, space="PSUM") as ps_dd,
            tc.tile_pool(name="ps_n", bufs=2, space="PSUM") as ps_n,
            tc.tile_pool(name="ps_o", bufs=2, space="PSUM") as ps_o,
        ):
            u_prev = None
            dinv_prev = None
            for gg in range(NG + 1):
                # stage 1: q projection + exp for group gg
                if gg < NG:
                    j0 = GRP * gg
                    yq = io2.tile([P, CT, GRP * P], F8, tag="yq")
                    nc.sync.dma_start(yq[:], ytb_v[:, :, j0 : j0 + GRP, :])
                    u = sb2.tile([P, CT, Ch], BF16, tag="u")
                    for kb in range(CT):
                        qk = ps_q.tile([P, Ch], F32, tag="qk")
                        for g in range(2):
                            nc.tensor.matmul(
                                qk[:],
                                wq8[:, g, :, kb, :],
                                yq[:, 2 * g : 2 * g + 2, :],
                                start=(g == 0),
                                stop=(g == 1),
                                perf_mode=DR,
                            )
                        nc.scalar.activation(
                            u[:, kb, :], qk[:], AF.Exp, bias=bq_col[:, kb : kb + 1]
                        )

                # stage 2: denB broadcast, attend, normalize for group gg-1
                if gg > 0:
                    att = sb2.tile([P, CT, Ch], BF16, tag="att")
                    for t in range(CT):
                        db = ps_dd.tile([P, Ch], F32, tag="dd")
                        nc.tensor.matmul(
                            db[:], selT[:, t, :], dinv_prev[:], start=True, stop=True
                        )
                        dbs = sb2.tile([P, Ch], BF16, tag="dbs")
                        nc.scalar.copy(dbs[:], db[:])
                        num = ps_n.tile([P, Ch], F32, tag="num")
                        nc.tensor.matmul(
                            num[:],
                            ctxR[:, t, :],
                            u_prev[:, t, :],
                            start=True,
                            stop=True,
                        )
                        nc.vector.tensor_mul(att[:, t, :], num[:], dbs[:])

                # stage 3: den sums + reciprocal (via scalar ln/exp) for group gg
                if gg < NG:
                    dden = ps_n.tile([P, Ch], F32, tag="num", name="dden")
                    for kb in range(CT):
                        nc.tensor.matmul(
                            dden[0:H, :],
                            sel_den[:, kb, :],
                            u[:, kb, :],
                            start=(kb == 0),
                            stop=(kb == CT - 1),
                        )
                    deninv = sb2.tile([H, Ch], BF16, tag="dinv")
                    with nc.allow_low_precision(reason="deninv rounded to bf16"):
                        nc.vector.reciprocal(deninv[:], dden[0:H, :])

                # stage 4: reproject + bias + store for group gg-1
                if gg > 0:
                    for j in range(GRP):
                        i = GRP * (gg - 1) + j
                        opre = ps_o.tile([P, Ch], F32, tag="opre")
                        for t in range(CT):
                            nc.tensor.matmul(
                                opre[:],
                                att[:, t, P * j : P * (j + 1)],
                                wr16[:, t, :],
                                start=(t == 0),
                                stop=(t == CT - 1),
                            )
                        o_sb = io2.tile([P, Ch], F32, tag="osb")
                        nc.vector.tensor_add(o_sb[:], opre[:], brb[:])
                        nc.sync.dma_start(out_d[P * i : P * (i + 1), :], o_sb[:])

                if gg < NG:
                    u_prev = u
                    dinv_prev = deninv

    nc.finalize()
    return nc


def _host_consts(Wk, bk, Wq, bq, Wv, bv, Wr, br):
    def w8(w):
        # [p, g, i, o] = W[256g + 128i + p, o]
        return np.ascontiguousarray(
            w.reshape(2, 2, P, Ch).transpose(2, 0, 1, 3).reshape(P, 4 * Ch)
        ).astype(F8_NP)

    # wq8: [p, g, i, kb, m] = Wq[256g + 128i + p, 128kb + m]
    wq8 = np.ascontiguousarray(
        Wq.reshape(2, 2, P, CT, P).transpose(2, 0, 1, 3, 4).reshape(P, 4 * Ch)
    ).astype(F8_NP)
    wr16 = np.ascontiguousarray(
        Wr.reshape(CT, P, Ch).transpose(1, 0, 2).reshape(P, CT * Ch)
    ).astype(BF16_NP)
    bq_col = np.ascontiguousarray(bq.reshape(CT, P).T).astype(np.float32)

    sel_den = np.zeros((P, CT, H), np.float32)
    for kb in range(CT):
        sel_den[0:64, kb, 2 * kb] = 1.0
        sel_den[64:128, kb, 2 * kb + 1] = 1.0

    selT_bc = np.zeros((H, CT, P), np.float32)
    for t in range(CT):
        selT_bc[2 * t, t, 0:64] = 1.0
        selT_bc[2 * t + 1, t, 64:128] = 1.0

    bvb = np.zeros((P, Ch), np.float32)
    for t in range(CT):
        for blk in range(2):
            p0 = 64 * blk
            c0 = P * t + p0
            bvb[p0 : p0 + 64, c0 : c0 + 64] = bv[None, c0 : c0 + 64]

    return {
        "wk8": w8(Wk),
        "wv8": w8(Wv),
        "wq8": wq8,
        "wr16": wr16,
        "bq_col": bq_col,
        "sel_den": sel_den.reshape(P, CT * H).astype(BF16_NP),
        "selT_bc": np.ascontiguousarray(selT_bc.reshape(H, CT * P)).astype(BF16_NP),
        "bv_blk": bvb,
        "br_bcast": np.ascontiguousarray(np.tile(br[None, :], (P, 1))).astype(
            np.float32
        ),
        "ones_col": np.ones((P, 2 * CT * 2), BF16_NP),
    }


_NC_CACHE = {}


def _get_nc():
    if "nc" not in _NC_CACHE:
        _NC_CACHE["nc"] = build_nc()
    return _NC_CACHE["nc"]


def kernel(input_, y, Wk, bk, Wq, bq, Wv, bv, Wr, br, _trace=False, _tmpdir=None):
    input_ = np.asarray(input_, np.float32)
    y = np.asarray(y, np.float32)
    consts = _host_consts(
        np.asarray(Wk, np.float32), np.asarray(bk, np.float32),
        np.asarray(Wq, np.float32), np.asarray(bq, np.float32),
        np.asarray(Wv, np.float32), np.asarray(bv, np.float32),
        np.asarray(Wr, np.float32), np.asarray(br, np.float32),
    )

    def tile_major(a8):
        # [p, tile, blk, tok] = a[128*tile+tok, 128*blk+p]
        return np.ascontiguousarray(
            a8.reshape(NT, P, CT, P).transpose(3, 0, 2, 1).reshape(P, NT * CT * P)
        )

    def blk_major(a8):
        # [p, blk, tile, tok]
        return np.ascontiguousarray(
            a8.reshape(NT, P, CT, P).transpose(3, 2, 0, 1).reshape(P, CT * NT * P)
        )

    nc = _get_nc()
    in_maps = []
    for i in range(B):
        x8 = input_[i].astype(F8_NP)
        y8 = y[i].astype(F8_NP)
        in_maps.append(
            {
                "xT8": tile_major(x8),
                "yT8": tile_major(y8),
                "yT8b": blk_major(y8),
                **consts,
            }
        )
    res = run_bass_kernel_spmd(
        nc, in_maps, core_ids=list(range(B)), trace=_trace, tmpdir=_tmpdir
    )
    out = np.stack([res.results[i]["out"] for i in range(B)], axis=0)
    if _trace:
        return out, res
    return out


# revision 10
# speedup vs baseline: 1.2217x; 1.0144x over previous
"""Trainium2 Bass kernel for efficient-attention (nn_Attention_65532611003000).

Sharding: data-parallel over batch. B == n_cores == 8, so core i processes
batch element i end-to-end; no collectives are needed.

v6 design: host-transposed fp8 activations, fp8 DoubleRow projections,
k-major pass 2 software-pipelined by one group with a stall-free stage
order (reciprocal issued after the output adds), and the pass-1->pass-2
boundary overlapped (group 0's q projection runs under the ctx epilogue).

Host prep (part of kernel(), like the weight rearrangement): x and y are
transposed to channel-major, quantized to fp8 e4m3, and interleaved into
one [x|y] stream per token tile; weights are pre-quantized (Wk/Wv/Wq fp8,
Wr bf16) and pre-tiled.

Per-core math ([Nt, Ch] = [4096, 512] activations, H=8 heads, 64 ch/head):
  pass 1 (per 128-token tile):
    kpre = xT8'@Wk8 + yT8'@Wk8              # fp8 DoubleRow (256-deep)
    khat = bf16(exp(kpre))                  # bk drops out of token-softmax
    vpre = xT8'@Wv8                         # fp8 DoubleRow
    S_t += khat_t' @ [vpre_t | 1]           # bf16, per 128-ch block t
  epilogue: ctx_t = S_t * (1/Zk) + bv       # blockdiag per 2 heads
  pass 2 (per 512-token group, pipelined by one group):
    qpreT = Wq8' @ yT8                      # fp8 DoubleRow, transposed out
    u = bf16(exp(qpreT + bq))               # bq as per-partition bias
    den = sel' @ u                          # per-head token sums (matmul)
    denB_t = selT' @ bf16(1/den)            # broadcast to v partitions
    att_t = bf16((ctx_t^T @ u_t) * denB_t)  # unnormalized attend, then mul
    out_j = sum_t att_t[:, j]' @ Wr16_t + br
"""

import sys

sys.path.insert(0, "/opt/trn_rl_repo")

import numpy as np
import ml_dtypes
from contextlib import ExitStack

import concourse.bass as bass
import concourse.bacc as bacc
import concourse.mybir as mybir
import concourse.tile as tile
from concourse.bass_utils import run_bass_kernel_spmd

B, Nt, Ch = 8, 4096, 512
H, HK = 8, 64
P = 128            # token chunk rows / SBUF partitions
NT = Nt // P       # 32 token tiles
CT = Ch // P       # 4 channel blocks
GRP = 4            # pass-2 tiles per group (512 tokens)
NG = NT // GRP     # 8 groups

F32 = mybir.dt.float32
BF16 = mybir.dt.bfloat16
F8 = mybir.dt.float8e4
AF = mybir.ActivationFunctionType
DR = mybir.MatmulPerfMode.DoubleRow

BF16_NP = ml_dtypes.bfloat16
F8_NP = ml_dtypes.float8_e4m3


def build_nc():
    nc = bacc.Bacc(None)

    # xyT8: [p, tile, s(x=0/y=1), blk, tok] = {x,y}[128*tile+tok, 128*blk+p]
    # yT8b: [p, blk, tile, tok]  (pass-2, blk-major for 512-token groups)
    xy_d = nc.declare_dram_parameter("xyT8", [P, NT * 2 * CT * P], F8, isOutput=False)
    ytb_d = nc.declare_dram_parameter("yT8b", [P, CT * NT * P], F8, isOutput=False)
    wk_d = nc.declare_dram_parameter("wk8", [P, 4 * Ch], F8, isOutput=False)
    wv_d = nc.declare_dram_parameter("wv8", [P, 4 * Ch], F8, isOutput=False)
    wq_d = nc.declare_dram_parameter("wq8", [P, 4 * Ch], F8, isOutput=False)
    wr_d = nc.declare_dram_parameter("wr16", [P, CT * Ch], BF16, isOutput=False)
    bqc_d = nc.declare_dram_parameter("bq_col", [P, CT], F32, isOutput=False)
    seld_d = nc.declare_dram_parameter("sel_den", [P, CT * H], BF16, isOutput=False)
    selt_d = nc.declare_dram_parameter("selT_bc", [H, CT * P], BF16, isOutput=False)
    bvb_d = nc.declare_dram_parameter("bv_blk", [P, Ch], F32, isOutput=False)
    brb_d = nc.declare_dram_parameter("br_bcast", [P, Ch], F32, isOutput=False)
    ones_d = nc.declare_dram_parameter("ones_col", [P, CT * 2], BF16, isOutput=False)
    out_d = nc.declare_dram_parameter("out", [Nt, Ch], F32, isOutput=True)

    xy_v = xy_d[:].rearrange("p (t s b q) -> p t s b q", t=NT, s=2, b=CT)
    ytb_v = ytb_d[:].rearrange("p (b t q) -> p b t q", b=CT, t=NT)

    with tile.TileContext(nc) as tc, ExitStack() as ctx:
        const = ctx.enter_context(tc.tile_pool(name="const", bufs=1))

        wk8 = const.tile([P, 2, 2, Ch], F8)
        wv8 = const.tile([P, 2, 2, Ch], F8)
        wq8 = const.tile([P, 2, 2, CT, P], F8)
        wr16 = const.tile([P, CT, Ch], BF16)
        bq_col = const.tile([P, CT], F32)
        sel_den = const.tile([P, CT, H], BF16)
        selT = const.tile([H, CT, P], BF16)
        bvb = const.tile([P, Ch], F32)
        brb = const.tile([P, Ch], F32)
        ctxR = const.tile([P, CT, P], BF16)      # per-head ctx, blockdiag
        zkinv = const.tile([P, CT], F32)

        nc.sync.dma_start(wk8[:], wk_d[:].rearrange("p (g i o) -> p g i o", g=2, i=2))
        nc.sync.dma_start(wv8[:], wv_d[:].rearrange("p (g i o) -> p g i o", g=2, i=2))
        nc.sync.dma_start(
            wq8[:], wq_d[:].rearrange("p (g i kb m) -> p g i kb m", g=2, i=2, kb=CT)
        )
        nc.sync.dma_start(wr16[:], wr_d[:].rearrange("p (t o) -> p t o", t=CT))
        nc.sync.dma_start(bq_col[:], bqc_d[:])
        nc.sync.dma_start(sel_den[:], seld_d[:].rearrange("p (t h) -> p t h", t=CT))
        nc.sync.dma_start(selT[:], selt_d[:].rearrange("p (t m) -> p t m", t=CT))
        nc.sync.dma_start(bvb[:], bvb_d[:])
        nc.sync.dma_start(brb[:], brb_d[:])

        # pools shared by the pass-1 tail and pass-2 head (boundary overlap)
        io2 = ctx.enter_context(tc.tile_pool(name="io2", bufs=3))
        sb2 = ctx.enter_context(tc.tile_pool(name="sb2", bufs=2))
        ps_q = ctx.enter_context(tc.tile_pool(name="ps_q", bufs=2, space="PSUM"))

        def stage1(gg):
            """q projection + exp for group gg -> u"""
            j0 = GRP * gg
            yq = io2.tile([P, CT, GRP * P], F8, tag="yq", name="yq")
            nc.sync.dma_start(yq[:], ytb_v[:, :, j0 : j0 + GRP, :])
            u = sb2.tile([P, CT, Ch], BF16, tag="u", name="u")
            for kb in range(CT):
                qk = ps_q.tile([P, Ch], F32, tag="qk", name="qk")
                for g in range(2):
                    nc.tensor.matmul(
                        qk[:],
                        wq8[:, g, :, kb, :],
                        yq[:, 2 * g : 2 * g + 2, :],
                        start=(g == 0),
                        stop=(g == 1),
                        perf_mode=DR,
                    )
                nc.scalar.activation(
                    u[:, kb, :], qk[:], AF.Exp, bias=bq_col[:, kb : kb + 1]
                )
            return u

        # ---------------- pass 1: khat, v, S & Zk accumulation --------------
        with (
            tc.tile_pool(name="io1", bufs=4) as io1,
            tc.tile_pool(name="sb1", bufs=2) as sb1,
            tc.tile_pool(name="ps_k", bufs=1, space="PSUM") as ps_k,
            tc.tile_pool(name="ps_v", bufs=1, space="PSUM") as ps_v,
            tc.tile_pool(name="ps_s", bufs=1, space="PSUM") as ps_s,
        ):
            s_acc = [
                ps_s.tile([P, 130], F32, tag=f"sacc{t}", name=f"sacc{t}")
                for t in range(CT)
            ]
            v_aug_bufs = [
                sb1.tile([P, CT, 130], BF16, tag=f"vaug{n}", name=f"vaug{n}")
                for n in range(2)
            ]
            for n in range(2):
                nc.sync.dma_start(
                    v_aug_bufs[n][:, :, 128:130],
                    ones_d[:].rearrange("p (t c) -> p t c", t=CT),
                )

            for i in range(NT):
                xy = io1.tile([P, 2, CT, P], F8, tag="xy")
                nc.gpsimd.dma_start(xy[:], xy_v[:, i, :, :, :])

                kpre = ps_k.tile([P, Ch], F32, tag="kpre")
                for s in range(2):
                    for g in range(2):
                        nc.tensor.matmul(
                            kpre[:],
                            xy[:, s, 2 * g : 2 * g + 2, :],
                            wk8[:, g, :, :],
                            start=(s == 0 and g == 0),
                            stop=(s == 1 and g == 1),
                            perf_mode=DR,
                        )
                khat = sb1.tile([P, Ch], BF16, tag="khat", name="khat")
                nc.scalar.activation(khat[:], kpre[:], AF.Exp)

                vpre = ps_v.tile([P, Ch], F32, tag="vpre")
                for g in range(2):
                    nc.tensor.matmul(
                        vpre[:],
                        xy[:, 0, 2 * g : 2 * g + 2, :],
                        wv8[:, g, :, :],
                        start=(g == 0),
                        stop=(g == 1),
                        perf_mode=DR,
                    )
                v_aug = v_aug_bufs[i % 2]
                nc.vector.tensor_copy(
                    v_aug[:, :, 0:128],
                    vpre[:].rearrange("p (t q) -> p t q", t=CT),
                )

                for t in range(CT):
                    nc.tensor.matmul(
                        s_acc[t][:],
                        khat[:, P * t : P * (t + 1)],
                        v_aug[:, t, :],
                        start=(i == 0),
                        stop=(i == NT - 1),
                    )

            # group 0's q projection overlaps the epilogue below
            u0 = stage1(0)

            # ------------- epilogue: ctx = S * zkinv + bv ------------------
            zk4 = sb1.tile([P, CT], F32, name="zk4", tag="zk4")
            for t in range(CT):
                nc.scalar.copy(zk4[:, t : t + 1], s_acc[t][:, 128:129])
            nc.vector.reciprocal(zkinv[:], zk4[:])
            for t in range(CT):
                nc.scalar.copy(ctxR[:, t, :], bvb[:, P * t : P * (t + 1)])
                for blk in range(2):
                    p0 = 64 * blk
                    nc.vector.scalar_tensor_tensor(
                        ctxR[p0 : p0 + 64, t, p0 : p0 + 64],
                        s_acc[t][p0 : p0 + 64, p0 : p0 + 64],
                        zkinv[p0 : p0 + 64, t : t + 1],
                        bvb[p0 : p0 + 64, P * t + p0 : P * t + p0 + 64],
                        op0=mybir.AluOpType.mult,
                        op1=mybir.AluOpType.add,
                    )

        # -------- pass 2: q softmax, attend, reproject (1-group pipeline) ----
        with (
            tc.tile_pool(name="ps_dd", bufs=2, space="PSUM") as ps_dd,
            tc.tile_pool(name="ps_n", bufs=2, space="PSUM") as ps_n,
            tc.tile_pool(name="ps_o", bufs=2, space="PSUM") as ps_o,
        ):
            u_prev = None
            dinv_prev = None
            for gg in range(NG + 1):
                # stage 1: q projection + exp for group gg (gg=0 hoisted above)
                u = None
                if gg == 0:
                    u = u0
                elif gg < NG:
                    u = stage1(gg)

                # stage 2: denB broadcast, attend, normalize for group gg-1
                if gg > 0:
                    att = sb2.tile([P, CT, Ch], BF16, tag="att")
                    for t in range(CT):
                        db = ps_dd.tile([P, Ch], F32, tag="dd")
                        nc.tensor.matmul(
                            db[:], selT[:, t, :], dinv_prev[:], start=True, stop=True
                        )
                        dbs = sb2.tile([P, Ch], BF16, tag="dbs")
                        nc.scalar.copy(dbs[:], db[:])
                        num = ps_n.tile([P, Ch], F32, tag="num")
                        nc.tensor.matmul(
                            num[:],
                            ctxR[:, t, :],
                            u_prev[:, t, :],
                            start=True,
                            stop=True,
                        )
                        nc.vector.tensor_mul(att[:, t, :], num[:], dbs[:])

                # stage 4: reproject + bias + store for group gg-1
                if gg > 0:
                    for j in range(GRP):
                        i = GRP * (gg - 1) + j
                        opre = ps_o.tile([P, Ch], F32, tag="opre")
                        for t in range(CT):
                            nc.tensor.matmul(
                                opre[:],
                                att[:, t, P * j : P * (j + 1)],
                                wr16[:, t, :],
                                start=(t == 0),
                                stop=(t == CT - 1),
                            )
                        o_sb = io2.tile([P, Ch], F32, tag="osb")
                        nc.vector.tensor_add(o_sb[:], opre[:], brb[:])
                        nc.sync.dma_start(out_d[P * i : P * (i + 1), :], o_sb[:])

                # stage 3: den sums + reciprocal for group gg (after the adds
                # so the reciprocal never delays them on the vector queue)
                if gg < NG:
                    dden = ps_n.tile([P, Ch], F32, tag="num", name="dden")
                    for kb in range(CT):
                        nc.tensor.matmul(
                            dden[0:H, :],
                            sel_den[:, kb, :],
                            u[:, kb, :],
                            start=(kb == 0),
                            stop=(kb == CT - 1),
                        )
                    deninv = sb2.tile([H, Ch], BF16, tag="dinv")
                    with nc.allow_low_precision(reason="deninv rounded to bf16"):
                        nc.vector.reciprocal(deninv[:], dden[0:H, :])

                if gg < NG:
                    u_prev = u
                    dinv_prev = deninv

    nc.finalize()
    return nc


def _host_consts(Wk, bk, Wq, bq, Wv, bv, Wr, br):
    def w8(w):
        # [p, g, i, o] = W[256g + 128i + p, o]
        return np.ascontiguousarray(
            w.reshape(2, 2, P, Ch).transpose(2, 0, 1, 3).reshape(P, 4 * Ch)
        ).astype(F8_NP)

    # wq8: [p, g, i, kb, m] = Wq[256g + 128i + p, 128kb + m]
    wq8 = np.ascontiguousarray(
        Wq.reshape(2, 2, P, CT, P).transpose(2, 0, 1, 3, 4).reshape(P, 4 * Ch)
    ).astype(F8_NP)
    wr16 = np.ascontiguousarray(
        Wr.reshape(CT, P, Ch).transpose(1, 0, 2).reshape(P, CT * Ch)
    ).astype(BF16_NP)
    bq_col = np.ascontiguousarray(bq.reshape(CT, P).T).astype(np.float32)

    sel_den = np.zeros((P, CT, H), np.float32)
    for kb in range(CT):
        sel_den[0:64, kb, 2 * kb] = 1.0
        sel_den[64:128, kb, 2 * kb + 1] = 1.0

    selT_bc = np.zeros((H, CT, P), np.float32)
    for t in range(CT):
        selT_bc[2 * t, t, 0:64] = 1.0
        selT_bc[2 * t + 1, t, 64:128] = 1.0

    bvb = np.zeros((P, Ch), np.float32)
    for t in range(CT):
        for blk in range(2):
            p0 = 64 * blk
            c0 = P * t + p0
            bvb[p0 : p0 + 64, c0 : c0 + 64] = bv[None, c0 : c0 + 64]

    return {
        "wk8": w8(Wk),
        "wv8": w8(Wv),
        "wq8": wq8,
        "wr16": wr16,
        "bq_col": bq_col,
        "sel_den": sel_den.reshape(P, CT * H).astype(BF16_NP),
        "selT_bc": np.ascontiguousarray(selT_bc.reshape(H, CT * P)).astype(BF16_NP),
        "bv_blk": bvb,
        "br_bcast": np.ascontiguousarray(np.tile(br[None, :], (P, 1))).astype(
            np.float32
        ),
        "ones_col": np.ones((P, CT * 2), BF16_NP),
    }


_NC_CACHE = {}


def _get_nc():
    if "nc" not in _NC_CACHE:
        _NC_CACHE["nc"] = build_nc()
    return _NC_CACHE["nc"]


def kernel(input_, y, Wk, bk, Wq, bq, Wv, bv, Wr, br, _trace=False, _tmpdir=None):
    input_ = np.asarray(input_, np.float32)
    y = np.asarray(y, np.float32)
    consts = _host_consts(
        np.asarray(Wk, np.float32), np.asarray(bk, np.float32),
        np.asarray(Wq, np.float32), np.asarray(bq, np.float32),
        np.asarray(Wv, np.float32), np.asarray(bv, np.float32),
        np.asarray(Wr, np.float32), np.asarray(br, np.float32),
    )

    nc = _get_nc()
    in_maps = []
    for i in range(B):
        x8 = input_[i].astype(F8_NP)
        y8 = y[i].astype(F8_NP)
        # [p, tile, s, blk, tok]
        xT = x8.reshape(NT, P, CT, P).transpose(3, 0, 2, 1)
        yT = y8.reshape(NT, P, CT, P).transpose(3, 0, 2, 1)
        xy = np.ascontiguousarray(
            np.stack([xT, yT], axis=2).reshape(P, NT * 2 * CT * P)
        )
        ytb = np.ascontiguousarray(
            y8.reshape(NT, P, CT, P).transpose(3, 2, 0, 1).reshape(P, CT * NT * P)
        )
        in_maps.append({"xyT8": xy, "yT8b": ytb, **consts})
    res = run_bass_kernel_spmd(
        nc, in_maps, core_ids=list(range(B)), trace=_trace, tmpdir=_tmpdir
    )
    out = np.stack([res.results[i]["out"] for i in range(B)], axis=0)
    if _trace:
        return out, res
    return out
